# revision 38
# baseline (speedup 1.0000x reference)
"""Trainium2 Bass kernel for DepthBranch: feat = relu(conv2(relu(conv1(x)))),
per-pixel argmin over depth hypotheses, one-hot scatter multiply into
(B, C, D, H, W) prior volume.

Sharding: 8 cores = (batch b in {0,1}) x (64-row H band q in {0..3}).
Each core computes its full band on-device and writes [D*C, 64*320] f32
(d-major rows); the host gather transposes back to (C, D, H, W).

Device algorithm per core:
  conv1 via PE matmuls (float32r moving operands); conv2 with the output
  channels replicated 4x along PSUM partitions (w2 columns tiled to 128)
  so the [128, W] PSUM rows are already the d-major fill pattern
  featrep[r, p] = feat[r % 32, p] -- the feat replication costs zero
  extra PE time.  Per-pixel argmin on ACT+DVE/GPSIMD (exact f32,
  first-tie semantics match jnp.argmin), idx broadcast via PE, and each
  output row-tile materialized with ONE fused DVE scalar_tensor_tensor
  (idx==d_of_row)*featrep over a whole [128, 5120] group, streamed to
  DRAM as 48 x 2.56 MB dma_starts on the SP/ACT HWDGE rings (20 KB
  per-partition descriptors, >=1 MiB high-efficiency DMA regime).
"""

import sys

for _p in ("/opt/trn_rl_repo", "/root/.axon_site/_ro/trn_rl_repo"):
    if _p not in sys.path:
        sys.path.insert(0, _p)

import numpy as np

import concourse.mybir as mybir
import concourse.tile as tile
from concourse.tile import add_dep_helper
from concourse import bacc
from concourse.bass_utils import run_bass_kernel_spmd

F32 = mybir.dt.float32
F32R = mybir.dt.float32r
ALU = mybir.AluOpType
ACTF = mybir.ActivationFunctionType

# Problem geometry (hardcoded per spec nn_DepthBranch_42580305772560)
B, H, W, D, C, C1 = 2, 256, 320, 48, 32, 16
BAND = 64                     # H rows per core
PIX = BAND * W                # 20480 pixels per core
R = 16                        # rows per processing group
G = BAND // R                 # 4 groups
GPIX = R * W                  # 5120 pixels per group
FCOLS = PIX // 128            # 160 pixel-major columns
GF = GPIX // 128              # 40 f-columns per group
HF = GF // 2                  # 20 f-columns per argmin half-chain
N = 1024                      # idx-broadcast chunk (pixels)
NCH = GPIX // N               # 5 chunks per group
NT = (C * D) // 128           # 12 output row-tiles of 128
BIG = 1000.0

_CACHE: dict = {}


def _build_nc(reps=1):
    nc = bacc.Bacc(None, target_bir_lowering=False)

    x9_d = nc.dram_tensor("x9", [9, 66 * 322], F32R, kind="ExternalInput")
    xpm_d = nc.dram_tensor("xpm", [128, FCOLS], F32, kind="ExternalInput")
    hypb_d = nc.dram_tensor("hypB", [128, D], F32, kind="ExternalInput")
    w1t_d = nc.dram_tensor("w1T", [9, C1], F32R, kind="ExternalInput")
    b1m_d = nc.dram_tensor("b1m", [C1, G * (R + 2)], F32, kind="ExternalInput")
    rmsk_d = nc.dram_tensor("rmask", [C1, G * (R + 2)], F32, kind="ExternalInput")
    w2t3_d = nc.dram_tensor("w2T3r", [3 * C1, 3 * 128], F32R, kind="ExternalInput")
    b2c_d = nc.dram_tensor("b2r", [128, 1], F32, kind="ExternalInput")
    dpat_d = nc.dram_tensor("dpat2", [128, NT], F32, kind="ExternalInput")
    iotb_d = nc.dram_tensor("iotaBIG", [128, D], F32, kind="ExternalInput")
    ident_d = nc.dram_tensor("ident", [128, 128], F32R, kind="ExternalInput")
    out_d = nc.dram_tensor("out", [C * D, PIX], F32, kind="ExternalOutput")

    with tile.TileContext(nc) as tc:
        with (
            tc.tile_pool(name="const", bufs=1) as constp,
            tc.tile_pool(name="x9p", bufs=1) as x9p,
            tc.tile_pool(name="x2p", bufs=1) as x2p,
            tc.tile_pool(name="featp", bufs=2) as featp,
            tc.tile_pool(name="argm", bufs=3) as argm,
            tc.tile_pool(name="argv", bufs=2) as argv,
            tc.tile_pool(name="idxp", bufs=4) as idxp,
            tc.tile_pool(name="idxbp", bufs=2) as idxbp,
            tc.tile_pool(name="outp", bufs=4) as outp,
            tc.tile_pool(name="psI", bufs=2, space="PSUM") as psI,
            tc.tile_pool(name="psC", bufs=3, space="PSUM") as psC,
        ):
            # --- load constants once, split across both HWDGE rings with
            # the conv1 chain (x9, w1t) and argmin chain (xpm, hypB) first so
            # both producer pipelines start as early as possible ---
            def ld(dram, shape, tag, dt=F32, eng=None):
                t = constp.tile(shape, dt, tag=tag)
                (eng or nc.sync).dma_start(out=t[:], in_=dram[:])
                return t

            xpm = ld(xpm_d, [128, FCOLS], "xpm")
            hypb = ld(hypb_d, [128, D], "hypb")
            x9_first = x9p.tile([9, (R + 2) * 322], F32R, tag="x9", name="x9_0")
            nc.sync.dma_start(out=x9_first[:], in_=x9_d[:, 0 : (R + 2) * 322])
            w1t = ld(w1t_d, [9, C1], "w1t", F32R)
            b1m = ld(b1m_d, [C1, G * (R + 2)], "b1m", eng=nc.scalar)
            rmsk = ld(rmsk_d, [C1, G * (R + 2)], "rmsk", eng=nc.scalar)
            iotb = ld(iotb_d, [128, D], "iotb", eng=nc.scalar)
            w2t3 = ld(w2t3_d, [3 * C1, 3 * 128], "w2t3", F32R, eng=nc.scalar)
            b2r = ld(b2c_d, [128, 1], "b2r", eng=nc.scalar)
            dpat = ld(dpat_d, [128, NT], "dpat", eng=nc.scalar)
            ident = ld(ident_d, [128, 128], "ident", F32R, eng=nc.scalar)

            feats = {}
            idxs = {}
            idxbs = {}
            x2s = {}
            conv2_last = {}

            def load_x9(g):
                r0 = R * g
                x9_g = x9p.tile([9, (R + 2) * 322], F32R, tag="x9", name=f"x9_{_rep}_{g}")
                nc.sync.dma_start(
                    out=x9_g[:], in_=x9_d[:, r0 * 322 : (r0 + R + 2) * 322]
                )
                return x9_g

            slabs_done = {}

            def emit_conv1(g, rho0, rho1, x9_g=None):
                # conv1: rows r0-1 .. r0+R (18 conv1-grid rows), emitted in
                # [rho0, rho1) sub-slabs so conv2/fill can start early
                if rho0 == 0:
                    if x9_g is None:
                        x9_g = load_x9(g)
                    x2s[g] = (
                        x2p.tile(
                            [3 * C1, R + 2, 322], F32R, tag="x2", name=f"x2_{_rep}_{g}"
                        ),
                        x9_g,
                    )
                    slabs_done[g] = set()
                    # out-of-image halo columns (image cols -1 and 320) are
                    # zero; written up-front so they don't serialize behind
                    # the relus
                    nc.gpsimd.memset(x2s[g][0][0:C1, :, 0:1].bitcast(F32), 0.0)
                    nc.gpsimd.memset(x2s[g][0][0:C1, :, 321:322].bitcast(F32), 0.0)
                x2_3, x9_g = x2s[g]
                for rho in range(rho0, rho1):
                    p1 = psC.tile([C1, 322], F32, tag="c", name=f"p1_{_rep}_{g}_{rho}")
                    nc.tensor.matmul(
                        p1[:],
                        w1t[:],
                        x9_g[:, rho * 322 : (rho + 1) * 322],
                        start=True,
                        stop=True,
                    )
                    col = g * (R + 2) + rho
                    nc.scalar.activation(
                        x2_3[0:C1, rho, 1:321],
                        p1[:, 1:321],
                        ACTF.Relu,
                        scale=rmsk[:, col : col + 1],
                        bias=b1m[:, col : col + 1],
                    )
                # dx-shifted partition copies for K=48 conv2 taps, in 6-row
                # slabs so conv2 rows can start before conv1 fully finishes
                for sl in range(3):
                    if sl not in slabs_done[g] and 6 * sl + 6 <= rho1:
                        slabs_done[g].add(sl)
                        for dx in (1, 2):
                            nc.gpsimd.dma_start(
                                out=x2_3[
                                    dx * C1 : (dx + 1) * C1, 6 * sl : 6 * sl + 6, 0:320
                                ],
                                in_=x2_3[0:C1, 6 * sl : 6 * sl + 6, dx : dx + 320],
                            )

            def emit_conv2(g, r0, r1):
                # conv2 with output channels tiled 4x along PSUM partitions:
                # featrep[r, p] = relu(conv2)[r % 32, p] lands directly in the
                # d-major fill layout.  3 accumulating K=48 matmuls per row.
                x2_3 = x2s[g][0]
                if r0 == 0:
                    feats[g] = featp.tile(
                        [128, GPIX], F32, tag="feat", name=f"feat_{_rep}_{g}"
                    )
                featrep = feats[g]
                for r in range(r0, r1):
                    p2 = psC.tile([128, W], F32, tag="c", name=f"p2_{_rep}_{g}_{r}")
                    for dy in range(3):
                        mm_i = nc.tensor.matmul(
                            p2[:],
                            w2t3[:, dy * 128 : (dy + 1) * 128],
                            x2_3[:, r + dy, 0:W],
                            start=(dy == 0),
                            stop=(dy == 2),
                        )
                        conv2_last[g] = mm_i
                    nc.scalar.activation(
                        featrep[:, r * W : (r + 1) * W], p2[:], ACTF.Relu, bias=b2r[:]
                    )

            def emit_argmin(g, fa, fb, after=None):
                # per-pixel argmin over D hypotheses (pixel-major, exact f32)
                # for group-fcols [fa, fb); shorter chains at the very head
                # deliver the first idx values earlier.
                f0 = g * GF + fa
                HFr = fb - fa
                h = fa
                draw = argm.tile([128, HFr, D], F32, tag="a3", name=f"draw_{_rep}_{g}_{h}")
                # diff[p,f,d] = hyp[d] - x[p,f] via dual broadcast APs on
                # idle GPSIMD, then |.| on ACT.  Exact f32.
                diff_i = nc.gpsimd.tensor_tensor(
                    out=draw[:],
                    in0=hypb[:]
                    .rearrange("p (o d) -> p o d", o=1)
                    .broadcast_to((128, HFr, D)),
                    in1=xpm[:, f0 : f0 + HFr]
                    .rearrange("p (f o) -> p f o", o=1)
                    .broadcast_to((128, HFr, D)),
                    op=ALU.subtract,
                )
                if after is not None:
                    # keep future argmin work from being scheduled ahead
                    # of the current group's fill stream
                    add_dep_helper(diff_i.ins, after.ins)
                diff = argm.tile([128, HFr, D], F32, tag="a3", name=f"diff_{_rep}_{g}_{h}")
                nc.scalar.activation(diff[:], draw[:], ACTF.Abs)
                minv = argv.tile([128, HFr], F32, tag="av", name=f"minv_{_rep}_{g}_{h}")
                nc.vector.tensor_reduce(
                    out=minv[:], in_=diff[:], axis=mybir.AxisListType.X, op=ALU.min
                )
                eq = argm.tile([128, HFr, D], F32, tag="a3", name=f"eq_{_rep}_{g}_{h}")
                nc.vector.tensor_tensor(
                    out=eq[:],
                    in0=diff[:],
                    in1=minv[:]
                    .rearrange("p (f o) -> p f o", o=1)
                    .broadcast_to((128, HFr, D)),
                    op=ALU.is_equal,
                )
                cand = argm.tile([128, HFr, D], F32, tag="a3", name=f"cand_{_rep}_{g}_{h}")
                nc.vector.scalar_tensor_tensor(
                    out=cand[:],
                    in0=eq[:],
                    scalar=-BIG,
                    in1=iotb[:]
                    .rearrange("p (o d) -> p o d", o=1)
                    .broadcast_to((128, HFr, D)),
                    op0=ALU.mult,
                    op1=ALU.add,
                )
                # idx in f32r (small integers -> exact) so the broadcast
                # matmul in the fill runs at full PE rate.
                idx_h = idxp.tile([128, HFr], F32R, tag="avr", name=f"idx_{_rep}_{g}_{h}")
                nc.vector.tensor_reduce(
                    out=idx_h[:], in_=cand[:], axis=mybir.AxisListType.X, op=ALU.min
                )
                idxs.setdefault(g, []).append((fa, fb, idx_h))

            def emit_idxb(g, c0, c1):
                # broadcast idx[pixel] to all 128 partitions: [128, GPIX]
                if c0 == 0:
                    idxbs[g] = idxbp.tile(
                        [128, GPIX], F32, tag="ib", name=f"ib_{_rep}_{g}"
                    )
                idxb = idxbs[g]
                for cch in range(c0, c1):
                    pi = psI.tile([128, N], F32, tag="pi", name=f"pi_{_rep}_{g}_{cch}")
                    for j in range(N // 128):
                        fc = cch * (N // 128) + j
                        for fa_, fb_, th_ in idxs[g]:
                            if fa_ <= fc < fb_:
                                idx_h, fl = th_, fc - fa_
                                break
                        # pi[:, 128j+p] = idx[p, fc] for all rows
                        pi_i = nc.tensor.matmul(
                            pi[:, j * 128 : (j + 1) * 128],
                            idx_h[:, fl : fl + 1].broadcast_to((128, 128)),
                            ident[:],
                            start=True,
                            stop=True,
                        )
                        if cch == 0 and g in conv2_last:
                            # keep the idx-broadcast matmuls (which wait on
                            # the argmin chain) from head-blocking conv work
                            # on the in-order PE queue
                            add_dep_helper(pi_i.ins, conv2_last[g].ins, sync=False)
                    nc.scalar.copy(out=idxb[:, cch * N : (cch + 1) * N], in_=pi[:])

            HGP = GPIX // 2  # 2560-pixel half-group fill/DMA unit
            QGP = GPIX // 4  # 1280-pixel quarter unit (pipeline head/tail)

            # column segments per group: narrow units at the very head (so
            # the first DMA fires before the full group-half is produced) and
            # at the very tail (shorter final STT+DMA drain)
            def segs(g):
                if g == 0:
                    return [(0, QGP), (QGP, QGP), (HGP, HGP)]
                if g == G - 1:
                    return [(0, HGP), (HGP, QGP), (HGP + QGP, QGP)]
                return [(0, HGP), (HGP, HGP)]

            for _rep in range(reps):
                # prologue: just enough of group 0 for the first 1024-pixel
                # unit; the rest emits after the first fill STT.  The chunk-0
                # idx broadcast sits between conv1 and conv2 on the in-order
                # PE queue so the fill's first deps all complete by ~28us.
                emit_argmin(0, 0, HF // 2)
                emit_argmin(0, HF // 2, HF)
                emit_conv1(0, 0, 8, x9_g=x9_first if _rep == 0 else None)
                emit_idxb(0, 0, 2)
                emit_conv2(0, 0, 4)
                emit_argmin(0, HF, GF)
                first_stt = None
                for g in range(G):
                    idxb = idxbs[g]
                    sg = segs(g)
                    S = len(sg) * NT
                    # producer emission points for the next group
                    pts = (14, 19, 24, 26, 29) if len(sg) == 3 else (2, 7, 13, 15, 18)
                    for si, (off, wd) in enumerate(sg):
                        for t in range(NT):
                            step = si * NT + t
                            # stagger producer work between fill units so no
                            # engine queue ever starves the DMA stream
                            if g == 0 and si == 0 and t == 1:
                                emit_conv1(0, 8, R + 2)
                                emit_conv2(0, 4, 8)
                                emit_idxb(0, 2, 3)
                                emit_conv2(0, 8, R)
                                emit_idxb(0, 3, NCH)
                            if g + 1 < G:
                                if step == pts[0]:
                                    emit_conv1(g + 1, 0, R + 2)
                                elif step == pts[1]:
                                    emit_conv2(g + 1, 0, R)
                                elif step == pts[2]:
                                    emit_argmin(g + 1, 0, HF, after=first_stt)
                                elif step == pts[3]:
                                    emit_argmin(g + 1, HF, GF, after=first_stt)
                                elif step == pts[4]:
                                    emit_idxb(g + 1, 0, NCH)
                            ot = outp.tile(
                                [128, wd], F32, tag="ot", name=f"ot_{_rep}_{g}_{si}_{t}"
                            )
                            stt_i = nc.vector.scalar_tensor_tensor(
                                out=ot[:],
                                in0=idxb[:, off : off + wd],
                                scalar=dpat[:, t : t + 1],
                                in1=feats[g][:, off : off + wd],
                                op0=ALU.is_equal,
                                op1=ALU.mult,
                            )
                            if step == 0:
                                first_stt = stt_i
                            # all output DMAs ride the SP HWDGE ring: the SP
                            # sequencer has no other steady-state work, so ACT
                            # relu/copy batches can never stall a DMA issue
                            nc.sync.dma_start(
                                out=out_d[
                                    t * 128 : (t + 1) * 128,
                                    g * GPIX + off : g * GPIX + off + wd,
                                ],
                                in_=ot[:],
                            )
    nc.compile()
    return nc


def _consts(w1, b1, w2, b2):
    w1T = np.ascontiguousarray(w1.reshape(C1, 9).T, dtype=np.float32)
    # w2T3[dx*16+cin, dy*32+co] = w2[co, cin, dy, dx]; then tile the output
    # channel block 4x along the output-partition axis (d-major replication)
    w2T3 = w2.transpose(3, 1, 2, 0).reshape(3 * C1, 3, C)
    w2T3r = np.ascontiguousarray(
        np.tile(w2T3, (1, 1, 4)).reshape(3 * C1, 3 * 128), dtype=np.float32
    )
    b2r = np.ascontiguousarray(np.tile(b2, 4).reshape(128, 1), dtype=np.float32)
    ii = np.arange(128)
    # row r of output tile t is global row gi = t*128 + r = d*32 + c
    # -> d = gi // 32 = 4*t + r//32
    dpat2 = np.stack([4 * t + ii // 32 for t in range(NT)], axis=1).astype(np.float32)
    iotb = np.tile((np.arange(D) + BIG).astype(np.float32)[None, :], (128, 1))
    ident = np.eye(128, dtype=np.float32)
    return dict(
        w1T=w1T, w2T3r=w2T3r, b2r=b2r,
        dpat2=np.ascontiguousarray(dpat2), iotaBIG=np.ascontiguousarray(iotb),
        ident=ident,
    )


def _in_maps(ref_init_depth, depth_hypotheses, w1, b1, w2, b2):
    consts = _consts(
        np.asarray(w1, np.float32), np.asarray(b1, np.float32),
        np.asarray(w2, np.float32), np.asarray(b2, np.float32),
    )
    x = np.asarray(ref_init_depth, np.float32)
    hyp = np.asarray(depth_hypotheses, np.float32)
    b1f = np.asarray(b1, np.float32)

    in_maps = []
    for k in range(8):
        b, q = k // 4, k % 4
        h0 = BAND * q
        xb = x[b, 0]  # (H, W)
        xp = np.zeros((BAND + 4, W + 4), np.float32)
        lo, hi = max(0, h0 - 2), min(H, h0 + BAND + 2)
        xp[lo - (h0 - 2) : hi - (h0 - 2), 2 : 2 + W] = xb[lo:hi]
        x9 = np.stack(
            [xp[dy : dy + BAND + 2, dx : dx + W + 2] for dy in range(3) for dx in range(3)]
        ).reshape(9, (BAND + 2) * (W + 2))
        band = xb[h0 : h0 + BAND].reshape(PIX)
        xpm = np.ascontiguousarray(band.reshape(FCOLS, 128).T)
        hypB = np.tile(hyp[b][None, :], (128, 1))
        # conv1-row validity mask: image row = h0 + R*g - 1 + rho
        m = np.zeros(G * (R + 2), np.float32)
        for g in range(G):
            for rho in range(R + 2):
                img = h0 + R * g - 1 + rho
                m[g * (R + 2) + rho] = 1.0 if 0 <= img < H else 0.0
        rmask = np.tile(m[None, :], (C1, 1))
        b1m = b1f.reshape(C1, 1) * rmask
        in_maps.append(
            dict(
                x9=np.ascontiguousarray(x9),
                xpm=xpm,
                hypB=np.ascontiguousarray(hypB),
                b1m=np.ascontiguousarray(b1m),
                rmask=np.ascontiguousarray(rmask),
                **consts,
            )
        )
    return in_maps


def kernel(ref_init_depth, depth_hypotheses, w1, b1, w2, b2):
    if "nc" not in _CACHE:
        _CACHE["nc"] = _build_nc()
    nc = _CACHE["nc"]

    in_maps = _in_maps(ref_init_depth, depth_hypotheses, w1, b1, w2, b2)

    import os
    trace = os.environ.get("BASS_TRACE", "0") == "1"
    trace_cores = None
    if os.environ.get("BASS_TRACE_ALL", "0") == "1":
        trace_cores = list(range(8))
    res = run_bass_kernel_spmd(
        nc, in_maps, core_ids=list(range(8)), trace=trace, trace_cores=trace_cores
    )
    _CACHE["last_results"] = res
    out = np.empty((B, C, D, H, W), np.float32)
    for k in range(8):
        b, q = k // 4, k % 4
        # device rows are d-major (row = d*32 + c): transpose back to (C, D)
        out[b, :, :, BAND * q : BAND * (q + 1), :] = (
            res.results[k]["out"].reshape(D, C, BAND, W).transpose(1, 0, 2, 3)
        )
    return out
